# revision 21
# baseline (speedup 1.0000x reference)
"""Trainium2 Bass kernel for CAMPseudoLabel.

Math (from the reference):
  cam    = relu(feature[:, 1:] / 96**3);  cam = cam * (cam > 0.2)
  pseudo = argmax(cam, axis=1) (first occurrence), int32
  upd    = masks + pseudo * (masks == 0)
  dom    = tiny 2-layer conv3d classifier + linear on dom_feat

Sharding: 8 cores = batch(2) x depth-quarters(4) of the 96^3 volume.
Per core: feature slice [13, 24, 96, 96] -> [13, 128, 1728] (channel 0 of the
original 14 is dropped by the reference before any use, so it is never sent).

The CAM stream is the bulk of the work and is memory/DVE-bound: the exact-f32
thresholded cam is one fused DVE op per channel, while the argmax runs on a
bf16 key bf16(relu(s-gamma)) — an order-isomorphic transform of the
thresholded cam (monotone downcast; exact ties preserved) — which doubles DVE
throughput. masks/pseudo/upd travel as uint8 (values 0..25).

The dom classifier's conv1 weights are pre-transposed on the host into matmul
lhsT layout and sharded across cores by output channel (1/8 each); the conv1
partials are AllGathered (64KB) early, then every core finishes conv2/linear
redundantly (conv2 weights replicated: 0.88MB is cheaper than a second
collective's latency). Core 0's dom output is returned.
"""

import numpy as np

import concourse.bacc as bacc
import concourse.tile as tile
from concourse import mybir
from concourse.bass_utils import run_bass_kernel_spmd

F32 = mybir.dt.float32
I32 = mybir.dt.int32
U8 = mybir.dt.uint8
BF16 = mybir.dt.bfloat16
Alu = mybir.AluOpType
Act = mybir.ActivationFunctionType

P = 128                 # SBUF partitions
FT = 1728               # free size per core: 24*96*96 / 128
NCH = 13                # cam channels (original channels 1..13)
GAMMA = 0.2
SCALE = float(np.float32(1.0) / np.float32(96 ** 3))  # f32(1/VOL), as jnp computes it
N_CORES = 8


def build_program():
    nc = bacc.Bacc("TRN2", target_bir_lowering=False, debug=False)

    # --- DRAM I/O (per core) ---
    feat_d = nc.declare_dram_parameter("feat", [NCH, P, FT], F32, isOutput=False)
    masks_d = nc.declare_dram_parameter("masks", [P, FT], U8, isOutput=False)
    domf_d = nc.declare_dram_parameter("dom_feat", [512, 64], F32, isOutput=False)
    # host-pretransposed conv weights, lhsT layout [k, (icb,) ic, oc];
    # conv1 is oc-sharded per core (16 of 128), conv2 replicated
    c1wT_d = nc.declare_dram_parameter("conv1_wT", [27, 2, 128, 16], F32, isOutput=False)
    c1b_d = nc.declare_dram_parameter("conv1_b", [128, 1], F32, isOutput=False)
    c2wT_d = nc.declare_dram_parameter("conv2_wT", [27, 128, 64], F32, isOutput=False)
    c2b_d = nc.declare_dram_parameter("conv2_b", [64, 1], F32, isOutput=False)
    lw_d = nc.declare_dram_parameter("lin_w", [64, 1], F32, isOutput=False)
    lb_d = nc.declare_dram_parameter("lin_b", [1, 1], F32, isOutput=False)

    cam_d = nc.declare_dram_parameter("cam", [NCH, P, FT], F32, isOutput=True)
    pseudo_d = nc.declare_dram_parameter("pseudo", [P, FT], U8, isOutput=True)
    upd_d = nc.declare_dram_parameter("upd", [P, FT], U8, isOutput=True)
    dom_d = nc.declare_dram_parameter("dom", [2, 1], F32, isOutput=True)

    from contextlib import ExitStack
    with tile.TileContext(nc) as tc, ExitStack() as ctx:
        dom_st = build_dom_head(nc, tc, ctx, domf_d, c1wT_d, lb_d)
        hooks = {
            1: lambda: build_dom_weights2(nc, tc, dom_st, c2wT_d, c1b_d,
                                          c2b_d, lw_d),
            7: lambda: build_dom_mid(nc, tc, dom_st),
            10: lambda: build_dom_tail(nc, tc, dom_st, dom_d),
        }
        build_cam(nc, tc, feat_d, masks_d, cam_d, pseudo_d, upd_d, hooks=hooks)
    nc.finalize()
    return nc


def build_cam(nc, tc, feat_d, masks_d, cam_d, pseudo_d, upd_d, hooks=None):
    with (
        tc.tile_pool(name="featp", bufs=5) as featp,
        tc.tile_pool(name="sp", bufs=4) as sp,
        tc.tile_pool(name="thp", bufs=3) as thp,
        tc.tile_pool(name="kbp", bufs=3) as kbp,
        tc.tile_pool(name="gtp", bufs=2) as gtp,
        tc.tile_pool(name="state", bufs=1) as state,
    ):
        neg_gamma = state.tile([P, 1], F32)
        nc.gpsimd.memset(neg_gamma[:], -GAMMA)

        # The argmax runs on a bf16 key kb = bf16(relu(s - gamma)), an
        # order-isomorphic transform of the thresholded cam (exact ties
        # preserved; the downcast is monotone). bf16 doubles DVE throughput.
        idx = state.tile([P, FT], BF16)   # running argmax (values 0..12, exact)
        nc.gpsimd.memset(idx[:], 0)
        best = state.tile([P, FT], BF16)  # running key max

        for c in range(NCH):
            ft = featp.tile([P, FT], F32)
            nc.sync.dma_start(out=ft[:], in_=feat_d[c])
            s = sp.tile([P, FT], F32)
            nc.scalar.activation(s[:], ft[:], Act.Copy, bias=0.0, scale=SCALE)
            th = thp.tile([P, FT], F32)
            # th = (s > gamma) * s   (thresholded cam channel, exact f32)
            nc.vector.scalar_tensor_tensor(
                out=th[:], in0=s[:], scalar=GAMMA, in1=s[:],
                op0=Alu.is_gt, op1=Alu.mult,
            )
            nc.sync.dma_start(out=cam_d[c], in_=th[:])
            if c == 0:
                nc.scalar.activation(best[:], ft[:], Act.Relu, bias=neg_gamma[:, 0:1], scale=SCALE)
            else:
                kb = kbp.tile([P, FT], BF16)
                nc.scalar.activation(kb[:], ft[:], Act.Relu, bias=neg_gamma[:, 0:1], scale=SCALE)
                gt = gtp.tile([P, FT], BF16)
                nc.vector.tensor_tensor(out=gt[:], in0=kb[:], in1=best[:], op=Alu.is_gt)
                # idx = max(idx, gt * c): strictly-greater keeps first occurrence.
                # (split ops: tensor_scalar runs 4x in bf16, tensor_tensor 2x —
                # a fused scalar_tensor_tensor would run 1x)
                nc.vector.tensor_scalar_mul(out=gt[:], in0=gt[:], scalar1=float(c))
                nc.vector.tensor_tensor(out=idx[:], in0=idx[:], in1=gt[:], op=Alu.max)
                if c < NCH - 1:
                    nc.vector.tensor_tensor(out=best[:], in0=best[:], in1=kb[:], op=Alu.max)
            if hooks and c in hooks:
                hooks[c]()

        # masks/pseudo/upd travel as uint8 (values 0..25); math in bf16
        pseudo8 = state.tile([P, FT], U8)
        nc.scalar.copy(pseudo8[:], idx[:])
        nc.sync.dma_start(out=pseudo_d[:], in_=pseudo8[:])

        mk8 = state.tile([P, FT], U8)
        nc.sync.dma_start(out=mk8[:], in_=masks_d[:])
        mkb = state.tile([P, FT], BF16)
        nc.scalar.copy(mkb[:], mk8[:])
        eqz = state.tile([P, FT], BF16)
        nc.vector.tensor_scalar(out=eqz[:], in0=mkb[:], scalar1=0.0, scalar2=None,
                                op0=Alu.is_equal)
        contrib = state.tile([P, FT], BF16)
        nc.vector.tensor_tensor(out=contrib[:], in0=eqz[:], in1=idx[:], op=Alu.mult)
        upd8 = state.tile([P, FT], U8)
        nc.vector.tensor_tensor(out=upd8[:], in0=mkb[:], in1=contrib[:], op=Alu.add)
        nc.sync.dma_start(out=upd_d[:], in_=upd8[:])


def build_dom_head(nc, tc, ctx, domf_d, c1wT_d, lb_d):
    """dom classifier front: this core's conv1 weight slice (442KB, first on
    the sync queue), padded inputs, 54 partial matmuls, and the conv1
    AllGather — all ahead of the CAM loop so the collective's cross-core
    rendezvous happens as early as possible."""
    st = {}
    dw = ctx.enter_context(tc.tile_pool(name="dw", bufs=1))
    dact = ctx.enter_context(tc.tile_pool(name="dact", bufs=1))
    ddf = ctx.enter_context(tc.tile_pool(name="ddf", bufs=4))
    dpsum = ctx.enter_context(tc.tile_pool(name="dpsum", bufs=1, space="PSUM"))
    ddram = ctx.enter_context(tc.tile_pool(name="ddram", bufs=1, space="DRAM"))
    st["dact"], st["dpsum"], st["ddram"], st["dw"] = dact, dpsum, ddram, dw
    st["groups"] = [list(range(N_CORES))]
    st["lb_d"] = lb_d

    ones_t = dw.tile([P, 1], F32)
    nc.gpsimd.memset(ones_t[:], 1.0)
    st["ones"] = ones_t

    w1_s = dw.tile([128, 54 * 16], F32)
    nc.sync.dma_start(
        out=w1_s.rearrange("p (k i o) -> p k i o", k=27, i=2, o=16)[:],
        in_=c1wT_d.rearrange("k i p o -> p k i o"),
    )

    # padded conv1 input: [128, 6*6*6*2], (x, y, z, b) with batch innermost so
    # (z, b) merge into one AP dim and both batches share each matmul
    pad1 = {}
    for icb in range(2):
        pt = dact.tile([128, 432], F32, tag=f"pad1_{icb}", name=f"pad1_{icb}")
        nc.gpsimd.memset(pt[:], 0.0)
        for b in range(2):
            df = ddf.tile([128, 64], F32, tag="df", name=f"df_{b}_{icb}")
            nc.sync.dma_start(
                out=df[:],
                in_=domf_d[b * 256 + icb * 128 : b * 256 + icb * 128 + 128, :],
            )
            nc.scalar.copy(
                pt.rearrange("p (x y z b) -> p x y z b", x=6, y=6, z=6, b=2)[
                    :, 1:5, 1:5, 1:5, b
                ],
                df.rearrange("p (x y z) -> p x y z", x=4, y=4, z=4)[:],
            )
        pad1[icb] = pt

    # conv1 partial: 54 accumulating matmuls -> PSUM [oc_slice=16, (x,y,z,b)=128]
    psum_c1 = dpsum.tile([16, 128], F32)
    n_acc = 0
    for k in range(27):
        dx, dy, dz = k // 9, (k // 3) % 3, k % 3
        for icb in range(2):
            pv3 = pad1[icb].rearrange("p (x y zb) -> p x y zb", x=6, y=6, zb=12)
            nc.tensor.matmul(
                psum_c1[:],
                lhsT=w1_s[:, (k * 2 + icb) * 16 : (k * 2 + icb + 1) * 16],
                rhs=pv3[:, dx : dx + 4, dy : dy + 4, 2 * dz : 2 * dz + 8],
                start=(n_acc == 0),
                stop=(n_acc == 53),
            )
            n_acc += 1

    # evac (DVE, ~0.2us, first in its stream) + AllGather on the gpsimd queue
    sb1 = dact.tile([16, 128], F32, tag="sb1")
    nc.vector.tensor_copy(sb1[:], psum_c1[:])
    ccin1 = ddram.tile([16, 128], F32, tag="ccin1")
    nc.gpsimd.dma_start(out=ccin1[:], in_=sb1[:])
    ccout1 = ddram.tile([128, 128], F32, tag="ccout1", addr_space="Shared")
    nc.gpsimd.collective_compute(
        "AllGather", Alu.bypass, replica_groups=st["groups"],
        ins=[ccin1[:]], outs=[ccout1[:]],
    )
    full1 = dact.tile([128, 128], F32, tag="full1")
    nc.gpsimd.dma_start(out=full1[:], in_=ccout1[:])
    st["full1"] = full1
    return st


def build_dom_weights2(nc, tc, st, c2wT_d, c1b_d, c2b_d, lw_d):
    """conv2 weights (full 0.88MB, replicated: cheaper than a second
    collective's latency) + biases, issued after the first feat channel."""
    dw = st["dw"]
    w2_s = dw.tile([128, 27 * 64], F32, name="w2_s")
    nc.sync.dma_start(
        out=w2_s.rearrange("p (k o) -> p k o", k=27, o=64)[:],
        in_=c2wT_d.rearrange("k p o -> p k o"),
    )
    st["w2"] = w2_s
    for nm, d_, rows in (("c1b", c1b_d, 128), ("c2b", c2b_d, 64), ("lw", lw_d, 64)):
        t = dw.tile([rows, 1], F32, name=f"t_{nm}")
        nc.gpsimd.dma_start(out=t[:], in_=d_[:])
        st[nm] = t


def build_dom_mid(nc, tc, st):
    """Pools on the gathered conv1 + full conv2 (emitted mid CAM loop; inputs
    are ready well before these ops' stream positions execute)."""
    dact, dpsum = st["dact"], st["dpsum"]
    full1 = st["full1"]

    pad2 = dact.tile([128, 128], F32, tag="pad2")
    nc.gpsimd.memset(pad2[:], 0.0)
    for b in range(2):
        # full1 col index = ((x*4+y)*4+z)*2+b; x = 2x'+a etc.
        pool_v = full1.rearrange(
            "p (x a y b2 z c b) -> p x a y b2 z c b",
            x=2, a=2, y=2, b2=2, z=2, c=2, b=2,
        )
        mp = dact.tile([128, 8], F32, tag=f"mp1_{b}", name=f"mp1_{b}")
        mv = mp.rearrange("p (x y z) -> p x y z", x=2, y=2, z=2)
        first = True
        for da in range(2):
            for db in range(2):
                for dc in range(2):
                    v = pool_v[:, :, da, :, db, :, dc, b]
                    if first:
                        nc.vector.tensor_copy(mv[:], v)
                        first = False
                    else:
                        nc.vector.tensor_tensor(out=mv[:], in0=mv[:], in1=v, op=Alu.max)
        relu1 = dact.tile([128, 8], F32, tag=f"relu1_{b}", name=f"relu1_{b}")
        nc.scalar.activation(relu1[:], mp[:], Act.Relu, bias=st["c1b"][:, 0:1], scale=1.0)
        nc.scalar.copy(
            pad2.rearrange("p (x y z b) -> p x y z b", x=4, y=4, z=4, b=2)[
                :, 1:3, 1:3, 1:3, b
            ],
            relu1.rearrange("p (x y z) -> p x y z", x=2, y=2, z=2)[:],
        )

    # conv2 (full): 27 accumulating matmuls -> PSUM [oc=64, (x,y,z,b)=16]
    psum_c2 = dpsum.tile([64, 16], F32)
    pv2 = pad2.rearrange("p (x y zb) -> p x y zb", x=4, y=4, zb=8)
    for k in range(27):
        dx, dy, dz = k // 9, (k // 3) % 3, k % 3
        nc.tensor.matmul(
            psum_c2[:],
            lhsT=st["w2"][:, k * 64 : (k + 1) * 64],
            rhs=pv2[:, dx : dx + 2, dy : dy + 2, 2 * dz : 2 * dz + 4],
            start=(k == 0),
            stop=(k == 26),
        )
    st["psum_c2"] = psum_c2


def build_dom_tail(nc, tc, st, dom_d):
    """maxpool over 8 voxels -> +bias -> relu -> linear."""
    dact, dpsum = st["dact"], st["dpsum"]
    psum_c2 = st["psum_c2"]
    xw = dact.tile([128, 2], F32, tag="xw")
    for b in range(2):
        mp2 = dact.tile([64, 1], F32, tag=f"mp2_{b}", name=f"mp2_{b}")
        nc.vector.tensor_reduce(
            out=mp2[:],
            in_=psum_c2.rearrange("p (v b) -> p v b", v=8, b=2)[:, :, b],
            axis=mybir.AxisListType.X, op=Alu.max,
        )
        x2 = dact.tile([64, 1], F32, tag=f"x2_{b}", name=f"x2_{b}")
        nc.scalar.activation(x2[:], mp2[:], Act.Relu, bias=st["c2b"][:, 0:1], scale=1.0)
        nc.vector.tensor_tensor(
            out=xw[0:64, b : b + 1], in0=x2[:], in1=st["lw"][:], op=Alu.mult
        )
    # bias row: xw[64, b] = lin_b
    nc.gpsimd.dma_start(out=xw[64:65, 0:1], in_=st["lb_d"][:])
    nc.gpsimd.dma_start(out=xw[64:65, 1:2], in_=st["lb_d"][:])

    psd = dpsum.tile([2, 1], F32, tag="psd")
    nc.tensor.matmul(psd[:], lhsT=xw[0:65, 0:2], rhs=st["ones"][0:65, 0:1], start=True, stop=True)
    dom_s = dact.tile([2, 1], F32, tag="dom_s")
    nc.scalar.copy(dom_s[:], psd[:])
    nc.sync.dma_start(out=dom_d[:], in_=dom_s[:])


_NC_CACHE = None


def _get_nc():
    global _NC_CACHE
    if _NC_CACHE is None:
        _NC_CACHE = build_program()
    return _NC_CACHE


def make_in_maps(inputs):
    feature = np.ascontiguousarray(np.asarray(inputs["feature"], dtype=np.float32))
    masks = np.ascontiguousarray(np.asarray(inputs["masks"], dtype=np.uint8))
    c1w = np.asarray(inputs["conv1_w"], np.float32).reshape(128, 256, 27)
    c2w = np.asarray(inputs["conv2_w"], np.float32).reshape(64, 128, 27)
    # lhsT layout [k, icb, ic, oc] / [k, ic, oc]
    c1wT = np.ascontiguousarray(c1w.transpose(2, 1, 0)).reshape(27, 2, 128, 128)
    c2wT = np.ascontiguousarray(c2w.transpose(2, 1, 0))
    shared = {
        "dom_feat": np.asarray(inputs["dom_feat"], np.float32).reshape(512, 64),
        "conv2_wT": c2wT,
        "conv1_b": np.asarray(inputs["conv1_b"], np.float32).reshape(128, 1),
        "conv2_b": np.asarray(inputs["conv2_b"], np.float32).reshape(64, 1),
        "lin_w": np.asarray(inputs["lin_w"], np.float32).reshape(64, 1),
        "lin_b": np.asarray(inputs["lin_b"], np.float32).reshape(1, 1),
    }
    in_maps = []
    for core in range(N_CORES):
        b, q = divmod(core, 4)
        dsl = slice(q * 24, (q + 1) * 24)
        in_maps.append(
            {
                "feat": np.ascontiguousarray(feature[b, 1:, dsl]).reshape(NCH, P, FT),
                "masks": np.ascontiguousarray(masks[b, 0, dsl]).reshape(P, FT),
                # this core's oc slice of conv1
                "conv1_wT": np.ascontiguousarray(c1wT[..., core * 16 : core * 16 + 16]),
                **shared,
            }
        )
    return in_maps


def assemble(results):
    cam = np.empty((2, NCH, 96, 96, 96), np.float32)
    pseudo = np.empty((2, 1, 96, 96, 96), np.int32)
    upd = np.empty((2, 1, 96, 96, 96), np.int32)
    for core in range(N_CORES):
        b, q = divmod(core, 4)
        dsl = slice(q * 24, (q + 1) * 24)
        r = results[core]
        cam[b, :, dsl] = np.asarray(r["cam"]).reshape(NCH, 24, 96, 96)
        pseudo[b, 0, dsl] = np.asarray(r["pseudo"]).reshape(24, 96, 96).astype(np.int32)
        upd[b, 0, dsl] = np.asarray(r["upd"]).reshape(24, 96, 96).astype(np.int32)
    dom = np.asarray(results[0]["dom"]).reshape(2, 1)
    return cam, pseudo, upd, dom


def _run(inputs, trace=False):
    nc = _get_nc()
    in_maps = make_in_maps(inputs)
    res = run_bass_kernel_spmd(nc, in_maps, list(range(N_CORES)), trace=trace)
    return assemble(res.results), res.exec_time_ns


def kernel(**inputs):
    out, _ = _run(inputs, trace=False)
    return out


# revision 22
# speedup vs baseline: 1.1490x; 1.1490x over previous
"""Trainium2 Bass kernel for CAMPseudoLabel.

Math (from the reference):
  cam    = relu(feature[:, 1:] / 96**3);  cam = cam * (cam > 0.2)
  pseudo = argmax(cam, axis=1) (first occurrence), int32
  upd    = masks + pseudo * (masks == 0)
  dom    = tiny 2-layer conv3d classifier + linear on dom_feat

Sharding: 8 cores = batch(2) x depth-quarters(4) of the 96^3 volume.
Per core: feature slice [13, 24, 96, 96] -> [13, 128, 1728] (channel 0 of the
original 14 is dropped by the reference before any use, so it is never sent).

The CAM stream is the bulk of the work and is memory/DVE-bound: the exact-f32
thresholded cam is one fused DVE op per channel, while the argmax runs on a
bf16 key bf16(relu(s-gamma)) — an order-isomorphic transform of the
thresholded cam (monotone downcast; exact ties preserved) — which doubles DVE
throughput. masks/pseudo/upd travel as uint8 (values 0..25).

The dom classifier's conv1 weights are pre-transposed on the host into matmul
lhsT layout and sharded across cores by output channel (1/8 each); the conv1
partials are AllGathered (64KB) early, then every core finishes conv2/linear
redundantly (conv2 weights replicated: 0.88MB is cheaper than a second
collective's latency). Core 0's dom output is returned.
"""

import numpy as np

import concourse.bacc as bacc
import concourse.tile as tile
from concourse import mybir
from concourse.bass_utils import run_bass_kernel_spmd

F32 = mybir.dt.float32
I32 = mybir.dt.int32
U8 = mybir.dt.uint8
BF16 = mybir.dt.bfloat16
Alu = mybir.AluOpType
Act = mybir.ActivationFunctionType

P = 128                 # SBUF partitions
FT = 1728               # free size per core: 24*96*96 / 128
NCH = 13                # cam channels (original channels 1..13)
GAMMA = 0.2
SCALE = float(np.float32(1.0) / np.float32(96 ** 3))  # f32(1/VOL), as jnp computes it
N_CORES = 8


def build_program():
    nc = bacc.Bacc("TRN2", target_bir_lowering=False, debug=False)

    # --- DRAM I/O (per core) ---
    feat_d = nc.declare_dram_parameter("feat", [NCH, P, FT], F32, isOutput=False)
    masks_d = nc.declare_dram_parameter("masks", [P, FT], U8, isOutput=False)
    domf_d = nc.declare_dram_parameter("dom_feat", [512, 64], F32, isOutput=False)
    # host-pretransposed conv weights, lhsT layout [k, (icb,) ic, oc];
    # conv1 is oc-sharded per core (16 of 128), conv2 replicated
    c1wT_d = nc.declare_dram_parameter("conv1_wT", [27, 2, 128, 128], F32, isOutput=False)
    c1b_d = nc.declare_dram_parameter("conv1_b", [128, 1], F32, isOutput=False)
    c2wT_d = nc.declare_dram_parameter("conv2_wT", [27, 128, 64], F32, isOutput=False)
    c2b_d = nc.declare_dram_parameter("conv2_b", [64, 1], F32, isOutput=False)
    lw_d = nc.declare_dram_parameter("lin_w", [64, 1], F32, isOutput=False)
    lb_d = nc.declare_dram_parameter("lin_b", [1, 1], F32, isOutput=False)

    cam_d = nc.declare_dram_parameter("cam", [NCH, P, FT], F32, isOutput=True)
    pseudo_d = nc.declare_dram_parameter("pseudo", [P, FT], U8, isOutput=True)
    upd_d = nc.declare_dram_parameter("upd", [P, FT], U8, isOutput=True)
    dom_d = nc.declare_dram_parameter("dom", [2, 1], F32, isOutput=True)

    from contextlib import ExitStack
    with tile.TileContext(nc) as tc, ExitStack() as ctx:
        dom_st = build_dom_head(nc, tc, ctx, domf_d, c1wT_d, lb_d)
        hooks = {0: lambda: build_dom_weights2(nc, tc, dom_st, c2wT_d, c1b_d,
                                               c2b_d, lw_d)}
        for j in range(6):
            hooks[j + 1] = (lambda jj: lambda: build_dom_w1_chunk(nc, tc, dom_st, jj))(j)
        hooks[7] = lambda: build_dom_conv1(nc, tc, dom_st)
        hooks[11] = lambda: build_dom_mid(nc, tc, dom_st)
        build_cam(nc, tc, feat_d, masks_d, cam_d, pseudo_d, upd_d, hooks=hooks,
                  tail=lambda: build_dom_tail(nc, tc, dom_st, dom_d))
    nc.finalize()
    return nc


def build_cam(nc, tc, feat_d, masks_d, cam_d, pseudo_d, upd_d, hooks=None, tail=None):
    with (
        tc.tile_pool(name="featp", bufs=5) as featp,
        tc.tile_pool(name="sp", bufs=4) as sp,
        tc.tile_pool(name="thp", bufs=3) as thp,
        tc.tile_pool(name="kbp", bufs=3) as kbp,
        tc.tile_pool(name="gtp", bufs=2) as gtp,
        tc.tile_pool(name="state", bufs=1) as state,
    ):
        neg_gamma = state.tile([P, 1], F32)
        nc.gpsimd.memset(neg_gamma[:], -GAMMA)

        # The argmax runs on a bf16 key kb = bf16(relu(s - gamma)), an
        # order-isomorphic transform of the thresholded cam (exact ties
        # preserved; the downcast is monotone). bf16 doubles DVE throughput.
        idx = state.tile([P, FT], BF16)   # running argmax (values 0..12, exact)
        nc.gpsimd.memset(idx[:], 0)
        best = state.tile([P, FT], BF16)  # running key max

        for c in range(NCH):
            ft = featp.tile([P, FT], F32)
            nc.sync.dma_start(out=ft[:], in_=feat_d[c])
            s = sp.tile([P, FT], F32)
            nc.scalar.activation(s[:], ft[:], Act.Copy, bias=0.0, scale=SCALE)
            th = thp.tile([P, FT], F32)
            # th = (s > gamma) * s   (thresholded cam channel, exact f32)
            nc.vector.scalar_tensor_tensor(
                out=th[:], in0=s[:], scalar=GAMMA, in1=s[:],
                op0=Alu.is_gt, op1=Alu.mult,
            )
            nc.sync.dma_start(out=cam_d[c], in_=th[:])
            if c == 0:
                nc.scalar.activation(best[:], ft[:], Act.Relu, bias=neg_gamma[:, 0:1], scale=SCALE)
            else:
                kb = kbp.tile([P, FT], BF16)
                nc.scalar.activation(kb[:], ft[:], Act.Relu, bias=neg_gamma[:, 0:1], scale=SCALE)
                gt = gtp.tile([P, FT], BF16)
                nc.vector.tensor_tensor(out=gt[:], in0=kb[:], in1=best[:], op=Alu.is_gt)
                # idx = max(idx, gt * c): strictly-greater keeps first occurrence.
                # (split ops: tensor_scalar runs 4x in bf16, tensor_tensor 2x —
                # a fused scalar_tensor_tensor would run 1x)
                nc.vector.tensor_scalar_mul(out=gt[:], in0=gt[:], scalar1=float(c))
                nc.vector.tensor_tensor(out=idx[:], in0=idx[:], in1=gt[:], op=Alu.max)
                if c < NCH - 1:
                    nc.vector.tensor_tensor(out=best[:], in0=best[:], in1=kb[:], op=Alu.max)
            if hooks and c in hooks:
                hooks[c]()

        # masks/pseudo/upd travel as uint8 (values 0..25); math in bf16
        pseudo8 = state.tile([P, FT], U8)
        nc.scalar.copy(pseudo8[:], idx[:])
        nc.sync.dma_start(out=pseudo_d[:], in_=pseudo8[:])

        mk8 = state.tile([P, FT], U8)
        nc.sync.dma_start(out=mk8[:], in_=masks_d[:])
        mkb = state.tile([P, FT], BF16)
        nc.scalar.copy(mkb[:], mk8[:])
        eqz = state.tile([P, FT], BF16)
        nc.vector.tensor_scalar(out=eqz[:], in0=mkb[:], scalar1=0.0, scalar2=None,
                                op0=Alu.is_equal)
        contrib = state.tile([P, FT], BF16)
        nc.vector.tensor_tensor(out=contrib[:], in0=eqz[:], in1=idx[:], op=Alu.mult)
        upd8 = state.tile([P, FT], U8)
        nc.vector.tensor_tensor(out=upd8[:], in0=mkb[:], in1=contrib[:], op=Alu.add)
        nc.sync.dma_start(out=upd_d[:], in_=upd8[:])
        if tail is not None:
            tail()


def build_dom_head(nc, tc, ctx, domf_d, c1wT_d, lb_d):
    """dom classifier front: pools + padded conv1 inputs only. Weights stream
    in as small chunks between the feat channels (no DMA bubble); conv1 runs
    late on the otherwise-idle PE."""
    st = {}
    dw = ctx.enter_context(tc.tile_pool(name="dw", bufs=1))
    dact = ctx.enter_context(tc.tile_pool(name="dact", bufs=1))
    ddf = ctx.enter_context(tc.tile_pool(name="ddf", bufs=4))
    dpsum = ctx.enter_context(tc.tile_pool(name="dpsum", bufs=1, space="PSUM"))
    st["dact"], st["dpsum"], st["dw"] = dact, dpsum, dw
    st["lb_d"] = lb_d
    st["c1wT_d"] = c1wT_d

    ones_t = dw.tile([P, 1], F32)
    nc.gpsimd.memset(ones_t[:], 1.0)
    st["ones"] = ones_t

    w1_s = dw.tile([128, 54 * 128], F32)
    st["w1"] = w1_s

    # padded conv1 input: [128, 6*6*6*2], (x, y, z, b) with batch innermost so
    # (z, b) merge into one AP dim and both batches share each matmul
    pad1 = {}
    for icb in range(2):
        pt = dact.tile([128, 432], F32, tag=f"pad1_{icb}", name=f"pad1_{icb}")
        nc.gpsimd.memset(pt[:], 0.0)
        for b in range(2):
            df = ddf.tile([128, 64], F32, tag="df", name=f"df_{b}_{icb}")
            nc.sync.dma_start(
                out=df[:],
                in_=domf_d[b * 256 + icb * 128 : b * 256 + icb * 128 + 128, :],
            )
            nc.scalar.copy(
                pt.rearrange("p (x y z b) -> p x y z b", x=6, y=6, z=6, b=2)[
                    :, 1:5, 1:5, 1:5, b
                ],
                df.rearrange("p (x y z) -> p x y z", x=4, y=4, z=4)[:],
            )
        pad1[icb] = pt
    st["pad1"] = pad1
    return st


def build_dom_w1_chunk(nc, tc, st, j, n_chunks=6):
    """One 1/6 chunk (~590KB) of the conv1 weights, interleaved between feat
    channels so the stream never sees a multi-us weight bubble."""
    per = 54 // n_chunks
    w1v = st["w1"].rearrange("p (q o) -> p q o", q=54)
    src_v = st["c1wT_d"].rearrange("k i p o -> p (k i) o")
    nc.sync.dma_start(
        out=w1v[:, j * per : (j + 1) * per, :],
        in_=src_v[:, j * per : (j + 1) * per, :],
    )


def build_dom_conv1(nc, tc, st):
    """conv1: 54 accumulating matmuls -> PSUM [oc=128, (x,y,z,b)=128]."""
    dpsum, pad1, w1_s = st["dpsum"], st["pad1"], st["w1"]
    psum_c1 = dpsum.tile([128, 128], F32)
    n_acc = 0
    for k in range(27):
        dx, dy, dz = k // 9, (k // 3) % 3, k % 3
        for icb in range(2):
            pv3 = pad1[icb].rearrange("p (x y zb) -> p x y zb", x=6, y=6, zb=12)
            nc.tensor.matmul(
                psum_c1[:],
                lhsT=w1_s[:, (k * 2 + icb) * 128 : (k * 2 + icb + 1) * 128],
                rhs=pv3[:, dx : dx + 4, dy : dy + 4, 2 * dz : 2 * dz + 8],
                start=(n_acc == 0),
                stop=(n_acc == 53),
            )
            n_acc += 1
    st["full1"] = psum_c1


def build_dom_weights2(nc, tc, st, c2wT_d, c1b_d, c2b_d, lw_d):
    """conv2 weights (full 0.88MB, replicated: cheaper than a second
    collective's latency) + biases, issued after the first feat channel."""
    dw = st["dw"]
    w2_s = dw.tile([128, 27 * 64], F32, name="w2_s")
    nc.sync.dma_start(
        out=w2_s.rearrange("p (k o) -> p k o", k=27, o=64)[:],
        in_=c2wT_d.rearrange("k p o -> p k o"),
    )
    st["w2"] = w2_s
    for nm, d_, rows in (("c1b", c1b_d, 128), ("c2b", c2b_d, 64), ("lw", lw_d, 64)):
        t = dw.tile([rows, 1], F32, name=f"t_{nm}")
        nc.gpsimd.dma_start(out=t[:], in_=d_[:])
        st[nm] = t


def build_dom_mid(nc, tc, st):
    """Pools on the gathered conv1 + full conv2 (emitted mid CAM loop; inputs
    are ready well before these ops' stream positions execute)."""
    dact, dpsum = st["dact"], st["dpsum"]
    full1 = st["full1"]

    pad2 = dact.tile([128, 128], F32, tag="pad2")
    nc.gpsimd.memset(pad2[:], 0.0)
    for b in range(2):
        # full1 col index = ((x*4+y)*4+z)*2+b; x = 2x'+a etc.
        pool_v = full1.rearrange(
            "p (x a y b2 z c b) -> p x a y b2 z c b",
            x=2, a=2, y=2, b2=2, z=2, c=2, b=2,
        )
        mp = dact.tile([128, 8], F32, tag=f"mp1_{b}", name=f"mp1_{b}")
        mv = mp.rearrange("p (x y z) -> p x y z", x=2, y=2, z=2)
        first = True
        for da in range(2):
            for db in range(2):
                for dc in range(2):
                    v = pool_v[:, :, da, :, db, :, dc, b]
                    if first:
                        nc.vector.tensor_copy(mv[:], v)
                        first = False
                    else:
                        nc.vector.tensor_tensor(out=mv[:], in0=mv[:], in1=v, op=Alu.max)
        relu1 = dact.tile([128, 8], F32, tag=f"relu1_{b}", name=f"relu1_{b}")
        nc.scalar.activation(relu1[:], mp[:], Act.Relu, bias=st["c1b"][:, 0:1], scale=1.0)
        nc.scalar.copy(
            pad2.rearrange("p (x y z b) -> p x y z b", x=4, y=4, z=4, b=2)[
                :, 1:3, 1:3, 1:3, b
            ],
            relu1.rearrange("p (x y z) -> p x y z", x=2, y=2, z=2)[:],
        )

    # conv2 (full): 27 accumulating matmuls -> PSUM [oc=64, (x,y,z,b)=16]
    psum_c2 = dpsum.tile([64, 16], F32)
    pv2 = pad2.rearrange("p (x y zb) -> p x y zb", x=4, y=4, zb=8)
    for k in range(27):
        dx, dy, dz = k // 9, (k // 3) % 3, k % 3
        nc.tensor.matmul(
            psum_c2[:],
            lhsT=st["w2"][:, k * 64 : (k + 1) * 64],
            rhs=pv2[:, dx : dx + 2, dy : dy + 2, 2 * dz : 2 * dz + 4],
            start=(k == 0),
            stop=(k == 26),
        )
    st["psum_c2"] = psum_c2


def build_dom_tail(nc, tc, st, dom_d):
    """maxpool over 8 voxels -> +bias -> relu -> linear."""
    dact, dpsum = st["dact"], st["dpsum"]
    psum_c2 = st["psum_c2"]
    xw = dact.tile([128, 2], F32, tag="xw")
    for b in range(2):
        mp2 = dact.tile([64, 1], F32, tag=f"mp2_{b}", name=f"mp2_{b}")
        nc.vector.tensor_reduce(
            out=mp2[:],
            in_=psum_c2.rearrange("p (v b) -> p v b", v=8, b=2)[:, :, b],
            axis=mybir.AxisListType.X, op=Alu.max,
        )
        x2 = dact.tile([64, 1], F32, tag=f"x2_{b}", name=f"x2_{b}")
        nc.scalar.activation(x2[:], mp2[:], Act.Relu, bias=st["c2b"][:, 0:1], scale=1.0)
        nc.vector.tensor_tensor(
            out=xw[0:64, b : b + 1], in0=x2[:], in1=st["lw"][:], op=Alu.mult
        )
    # bias row: xw[64, b] = lin_b
    nc.gpsimd.dma_start(out=xw[64:65, 0:1], in_=st["lb_d"][:])
    nc.gpsimd.dma_start(out=xw[64:65, 1:2], in_=st["lb_d"][:])

    psd = dpsum.tile([2, 1], F32, tag="psd")
    nc.tensor.matmul(psd[:], lhsT=xw[0:65, 0:2], rhs=st["ones"][0:65, 0:1], start=True, stop=True)
    dom_s = dact.tile([2, 1], F32, tag="dom_s")
    nc.scalar.copy(dom_s[:], psd[:])
    nc.sync.dma_start(out=dom_d[:], in_=dom_s[:])


_NC_CACHE = None


def _get_nc():
    global _NC_CACHE
    if _NC_CACHE is None:
        _NC_CACHE = build_program()
    return _NC_CACHE


def make_in_maps(inputs):
    feature = np.ascontiguousarray(np.asarray(inputs["feature"], dtype=np.float32))
    masks = np.ascontiguousarray(np.asarray(inputs["masks"], dtype=np.uint8))
    c1w = np.asarray(inputs["conv1_w"], np.float32).reshape(128, 256, 27)
    c2w = np.asarray(inputs["conv2_w"], np.float32).reshape(64, 128, 27)
    # lhsT layout [k, icb, ic, oc] / [k, ic, oc]
    c1wT = np.ascontiguousarray(c1w.transpose(2, 1, 0)).reshape(27, 2, 128, 128)
    c2wT = np.ascontiguousarray(c2w.transpose(2, 1, 0))
    shared = {
        "dom_feat": np.asarray(inputs["dom_feat"], np.float32).reshape(512, 64),
        "conv1_wT": c1wT,
        "conv2_wT": c2wT,
        "conv1_b": np.asarray(inputs["conv1_b"], np.float32).reshape(128, 1),
        "conv2_b": np.asarray(inputs["conv2_b"], np.float32).reshape(64, 1),
        "lin_w": np.asarray(inputs["lin_w"], np.float32).reshape(64, 1),
        "lin_b": np.asarray(inputs["lin_b"], np.float32).reshape(1, 1),
    }
    in_maps = []
    for core in range(N_CORES):
        b, q = divmod(core, 4)
        dsl = slice(q * 24, (q + 1) * 24)
        in_maps.append(
            {
                "feat": np.ascontiguousarray(feature[b, 1:, dsl]).reshape(NCH, P, FT),
                "masks": np.ascontiguousarray(masks[b, 0, dsl]).reshape(P, FT),
                **shared,
            }
        )
    return in_maps


def assemble(results):
    cam = np.empty((2, NCH, 96, 96, 96), np.float32)
    pseudo = np.empty((2, 1, 96, 96, 96), np.int32)
    upd = np.empty((2, 1, 96, 96, 96), np.int32)
    for core in range(N_CORES):
        b, q = divmod(core, 4)
        dsl = slice(q * 24, (q + 1) * 24)
        r = results[core]
        cam[b, :, dsl] = np.asarray(r["cam"]).reshape(NCH, 24, 96, 96)
        pseudo[b, 0, dsl] = np.asarray(r["pseudo"]).reshape(24, 96, 96).astype(np.int32)
        upd[b, 0, dsl] = np.asarray(r["upd"]).reshape(24, 96, 96).astype(np.int32)
    dom = np.asarray(results[0]["dom"]).reshape(2, 1)
    return cam, pseudo, upd, dom


def _run(inputs, trace=False):
    nc = _get_nc()
    in_maps = make_in_maps(inputs)
    res = run_bass_kernel_spmd(nc, in_maps, list(range(N_CORES)), trace=trace)
    return assemble(res.results), res.exec_time_ns


def kernel(**inputs):
    out, _ = _run(inputs, trace=False)
    return out
